# revision 2
# baseline (speedup 1.0000x reference)
"""Quanvolutional layer (nn_ConvGenQuantum) as a Trainium2 Bass kernel.

The reference applies, per 2x2 image patch (p0,p1,p2,p3), a fixed 4-qubit
circuit: RY(p_w) encoders, then a fixed 8-gate random layer with params
theta[0..4], then measures <Z_w>. Conjugating each Z_w through the circuit
(Heisenberg picture) and dropping Pauli strings containing Y (the encoded
state is real, so those have zero expectation) collapses the whole circuit
to a closed form:

    q0 = cos(p0 + theta0); q1 = cos(p1); q2 = cos(p2); q3 = cos(p3 + theta3)
    E0 = cos(theta4) * q0
    E1 = cos(theta1) * q0 * q1
    E2 = E1 * q2
    E3 = E2 * q3

(theta2 -- the RZ -- drops out entirely.) cos is evaluated via the
half-angle identity cos(p + B) = 1 - 2*sin((p + B)/2)^2 so the ScalarE Sin
argument p/2 + B/2 stays within the activation table's validated range
(plane 3 uses bias theta3 - pi, flipping its cosine's sign; the final
multiply absorbs it via (2u^2 - 1)).

Per-core dataflow (batch sharded 4096/8 = 512 images, pure data parallel):
DRAM I/O is fp16 (the 2e-2 tolerance leaves ~20x headroom over fp16's
~9e-4 end-to-end error; host converts), halving HBM traffic vs fp32.
The shard is processed in CHUNK_GS pipeline chunks; per chunk:
  ScalarE: 3 Sin ops (planes 0 / 1+2 via one affine view / 3) -> u tile
  DVE:     W012 = -2u^2 (one fused STT), w3n = +2u3^2, r0 = c1*(W0+1),
           E1 = (W1+1)*r0, E2 = (W2+1)*E1, E3 = (w3n-1)*E2
           (E writes interleave the 4 planes per patch for a dense
           output DMA)
  GpSimd:  E0 = c4*(W0+1) (idle engine, takes the leaf op)
All chunk input DMAs are issued on Sync up front so no input load ever
queues behind an output DMA's completion wait.
"""

import numpy as np

import concourse.bass as bass
import concourse.bacc as bacc
import concourse.tile as tile
from concourse import mybir
from concourse.bass_utils import run_bass_kernel_spmd

F32 = mybir.dt.float32
F16 = mybir.dt.float16
N_CORES = 8
B_TOTAL = 4096
ROWS = B_TOTAL // N_CORES       # images per core
PIX = 784                       # 28*28
CHUNK_GS = (1, 3)               # images-per-partition per pipeline chunk

LAST_RESULT = None              # BassKernelResults of the most recent run


def _build(th0: float, th1: float, th3: float, th4: float,
           chunk_gs=(1, 3)):
    """Build the per-core Bass program for an x shard of [ROWS, 784]."""
    # Skip the Bass-init all-engine barrier (it serializes the preamble for
    # ~1us); the built-in const tiles it guards are re-registered below via
    # TileContext-tracked memsets instead.
    orig_barrier = bass.Bass.all_engine_barrier
    bass.Bass.all_engine_barrier = lambda self, **kw: None
    try:
        nc = bacc.Bacc(None, target_bir_lowering=False, debug=False)
    finally:
        bass.Bass.all_engine_barrier = orig_barrier

    # Skip the Tile-exit semaphore clear + its extra barrier: the NEFF
    # runtime postamble already resets every HW semaphore between
    # iterations, so the Tile-side clear is redundant.
    nc.clear_and_free_semaphores = lambda sems: None

    s1 = float(np.cos(th1))
    s4 = float(np.cos(th4))
    # Sin biases per pixel plane: cos(p+B) via 1-2*Sin((p+B)/2)^2.
    # Plane 3 uses B = th3 - pi => computes -cos(p3+th3); sign folded into
    # E3 = (2*u3^2 - 1) * E2.
    sin_bias = [float(th0 / 2), 0.0, float((th3 - np.pi) / 2)]

    x = nc.declare_dram_parameter("x", [ROWS, PIX], F16, isOutput=False)
    out = nc.declare_dram_parameter("out", [ROWS, PIX], F16, isOutput=True)

    assert sum(chunk_gs) * 128 == ROWS
    add = mybir.AluOpType.add
    sub = mybir.AluOpType.subtract
    mult = mybir.AluOpType.mult
    SIN = mybir.ActivationFunctionType.Sin

    with tile.TileContext(nc) as tc:
        with tc.tile_pool(name="p", bufs=1) as pool:
            # Register activation-bias constants without an all-engine
            # barrier: gpsimd memsets inside the TileContext (the scheduler
            # adds the write->read semaphore to the consuming Sin).
            for i, val in enumerate(dict.fromkeys([0.0] + sin_bias)):
                t = nc.alloc_sbuf_tensor(f"const-bias-{i}", [128, 1], F32)
                nc.gpsimd.memset(t.ap(), val)
                nc.const_aps.aps[(F32, val)] = t.ap()

            # Dummy activation so walrus's ACT table load (~1.3us) runs
            # during the input DMA instead of blocking the first real Sin.
            warm = nc.alloc_sbuf_tensor("act-warm", [128, 1], F32)
            nc.scalar.activation(warm.ap(), nc.const_aps.aps[(F32, 0.0)],
                                 SIN, bias=0.0, scale=1.0)

            # All input DMAs up front on Sync: an in-DMA issued after an
            # out-DMA would queue behind that out-DMA's completion wait.
            xts, ovds = [], []
            row0 = 0
            for c, G in enumerate(chunk_gs):
                xv = x[row0:row0 + 128 * G, :].rearrange(
                    "(p g) m -> p (g m)", g=G)
                ovds.append(out[row0:row0 + 128 * G, :].rearrange(
                    "(p g) m -> p (g m)", g=G))
                row0 += 128 * G
                xt = pool.tile([128, G * PIX], F16, tag=f"x{c}")
                nc.sync.dma_start(out=xt[:, :], in_=xv)
                xts.append(xt)

            for c, G in enumerate(chunk_gs):
                Q = G * 196
                GA = 14 * G
                xt = xts[c]
                # image pixel (2r+b, 2c+d) at free offset g*784+r*56+b*28+c*2+d
                x6 = xt.rearrange("p (g a b c d) -> p g a b c d",
                                  g=G, a=14, b=2, c=14, d=2)

                # u planes in one tile, each plane a contiguous Q block:
                # [u0 | u1 | u2 | u3].
                ua = pool.tile([128, 4 * Q], F16, tag=f"ua{c}")
                u0v = ua[:, 0:Q].rearrange("p (g a c) -> p g a c",
                                           g=G, a=14, c=14)
                nc.scalar.activation(u0v, x6[:, :, :, 0, :, 0], SIN,
                                     bias=sin_bias[0], scale=0.5)
                # Planes 1,2 share bias 0 and their intra-patch offsets
                # {1, 28} form an affine pair (step 27 x 2), so ONE Sin op
                # covers both; the output view splits them into the two
                # contiguous blocks.
                x12 = xt.rearrange("p (ga cc) -> p ga cc", cc=56)[
                    :, :, 1:55].rearrange("p ga (j c) -> p ga j c",
                                          j=2)[:, :, :, 0:27:2]
                u12v = ua[:, Q:3 * Q].rearrange("p (j ga c) -> p ga j c",
                                                j=2, c=14)
                nc.scalar.activation(u12v, x12, SIN, bias=0.0, scale=0.5)
                u3v = ua[:, 3 * Q:4 * Q].rearrange("p (g a c) -> p g a c",
                                                   g=G, a=14, c=14)
                nc.scalar.activation(u3v, x6[:, :, :, 1, :, 1], SIN,
                                     bias=sin_bias[2], scale=0.5)

                # W = -2u^2 for planes 0,1,2 fused in one DVE op (unit
                # stride both sides -> 2x fp16 mode); plane 3 as +2u^2.
                w = pool.tile([128, 4 * Q], F16, tag=f"w{c}")
                nc.vector.scalar_tensor_tensor(
                    w[:, 0:3 * Q], ua[:, 0:3 * Q], -2.0, ua[:, 0:3 * Q],
                    op0=mult, op1=mult)
                nc.vector.scalar_tensor_tensor(
                    w[:, 3 * Q:4 * Q], ua[:, 3 * Q:4 * Q], 2.0,
                    ua[:, 3 * Q:4 * Q], op0=mult, op1=mult)

                # contiguous (ga, c) views per plane
                w0 = w[:, 0:Q].rearrange("p (ga c) -> p ga c", c=14)
                w1 = w[:, Q:2 * Q].rearrange("p (ga c) -> p ga c", c=14)
                w2 = w[:, 2 * Q:3 * Q].rearrange("p (ga c) -> p ga c", c=14)
                w3n = w[:, 3 * Q:4 * Q].rearrange("p (ga c) -> p ga c", c=14)

                # r0 = s1*(W0+1) (DVE tensor_scalar, 2-port mode)
                r0 = pool.tile([128, Q], F16, tag=f"r0{c}")
                r0v = r0.rearrange("p (ga c) -> p ga c", c=14)
                nc.vector.tensor_scalar(r0v, w0, 1.0, s1,
                                        op0=add, op1=mult)

                ot = pool.tile([128, G * PIX], F16, tag=f"o{c}")
                # output elem for patch (ga, c) plane w at ga*56 + c*4 + w
                ov4 = ot.rearrange("p (ga c w) -> p ga c w", c=14, w=4)
                oQ = [ov4[:, :, :, i] for i in range(4)]

                # E0 = s4*(W0+1) on the otherwise-idle GpSimd
                nc.gpsimd.tensor_scalar(oQ[0], w0, 1.0, s4,
                                        op0=add, op1=mult)
                # E1 = (W1+1)*r0
                nc.vector.scalar_tensor_tensor(oQ[1], w1, 1.0, r0v,
                                               op0=add, op1=mult)
                # E2 = (W2+1)*E1
                nc.vector.scalar_tensor_tensor(oQ[2], w2, 1.0, oQ[1],
                                               op0=add, op1=mult)
                # E3 = (2u3^2-1)*E2 = cos(p3+th3)*E2
                nc.vector.scalar_tensor_tensor(oQ[3], w3n, 1.0, oQ[2],
                                               op0=sub, op1=mult)

                nc.sync.dma_start(out=ovds[c], in_=ot[:, :])

    if not nc.is_finalized():
        nc.finalize()
    return nc


def kernel(x: np.ndarray, theta: np.ndarray, _trace: bool = False) -> np.ndarray:
    global LAST_RESULT
    th = np.asarray(theta, dtype=np.float64)
    nc = _build(th0=float(th[0]), th1=float(th[1]), th3=float(th[3]),
                th4=float(th[4]), chunk_gs=CHUNK_GS)

    xf = np.ascontiguousarray(
        np.asarray(x).reshape(B_TOTAL, PIX).astype(np.float16))
    in_maps = [{"x": xf[i * ROWS:(i + 1) * ROWS]} for i in range(N_CORES)]
    res = run_bass_kernel_spmd(nc, in_maps, core_ids=list(range(N_CORES)),
                               trace=_trace)
    LAST_RESULT = res
    out = np.concatenate([res.results[i]["out"] for i in range(N_CORES)],
                         axis=0)
    return np.ascontiguousarray(out.astype(np.float32))
